# revision 3
# baseline (speedup 1.0000x reference)
"""Trainium2 Bass kernel for nn_CMSWrite (scatter_memory) — fp16, DVE-only update.

Phase-2 redesign vs the PSUM/rank-1 variant: since alpha enters the update
as a PER-SLOT scalar, each slot-tile update is ONE DVE op

    out[p, :] = exp_score[p, tile] * VBC[p, :] + M_pre[p, :]

with
  - keep = (1-decay) folded into M/K on the HOST (M_pre = fp16(keep*M)),
  - g/Z folded into the exp scores post-AllReduce (one tiny [128,64]
    scale per level),
  - VBC/KBC = un-normalized v/k rows broadcast across partitions by the
    PE ONCE per level (pre-AR).
This eliminates the per-tile rank-1 matmuls (~600ns each, PE was 89%
busy), the PSUM round-trip, the exp-row transpose and its DRAM bounce.
Phase 2 is pure {DMA stream + one DVE op per tile}.

Other changes:
  - M/K streamed fp16 both directions (rel_l2 ~3e-4, gate is 2e-2);
    outputs fp16 in two separate DRAM tensors, host upcasts.
  - MLP weights uploaded bf16 (halves their DMA, 2x PE).
  - A DUMMY AllReduce fires at t~0 to absorb the ~57us inter-core launch
    skew + cold-collective cost; the real per-level ARs then complete
    ~2.5us after their last trigger.
  - Deep M prefetch (B_MI chunks) keeps the load queue busy through the
    AR0 wait.
  - MLP chain split into a score-critical part (z -> LN -> relu -> k ->
    k-broadcast; gates AR trigger) and a tail (gate/sigmoid, v/tanh,
    VBC/KBC broadcasts) interleaved into the score pass.
"""

import math
import numpy as np

L = 3
N = 65536
DLVL = 512
DK = 128
DZ = 128
NCORES = 8
S = N // NCORES          # 8192 slots per core
T = S // 128             # 64 slot-tiles of 128
SUB = 16                 # tile-columns per phase-2 chunk
NCH = T // SUB           # 8 chunks per level
NJOB = L * NCH           # 24 chunk jobs across the 3 levels
B_MI = 6                 # M-chunk prefetch depth (16KB/partition each)
PE_N = 8                 # M-tiles per chunk routed to the PE+ACT channel
EPS = 1e-5
THRESH = 0.1
SCALE = 1.0 / math.sqrt(DK)

_STATE = {}
SKIP_CC = False   # debug: replace AllReduce with local copy (wrong numerics)


def _build_bass():
    import concourse.bacc as bacc
    import concourse.tile as tile
    import concourse.mybir as mybir
    from concourse.masks import make_identity

    f32 = mybir.dt.float32
    f16 = mybir.dt.float16
    bf16 = mybir.dt.bfloat16
    A = mybir.AluOpType
    AF = mybir.ActivationFunctionType
    AX = mybir.AxisListType

    nc = bacc.Bacc("TRN2", target_bir_lowering=False, debug=False,
                   num_devices=NCORES)

    Mp = nc.dram_tensor("Mp", [L, S, DLVL], f16, kind="ExternalInput").ap()
    Kp = nc.dram_tensor("Kp", [L, S, DK], f16, kind="ExternalInput").ap()
    xcatT = nc.dram_tensor("xcatT", [L, 128, 14], bf16, kind="ExternalInput").ap()
    wevT = nc.dram_tensor("wevT", [L, 1792, 128], bf16, kind="ExternalInput").ap()
    wvalT = nc.dram_tensor("wvalT", [L, 128, DLVL], bf16, kind="ExternalInput").ap()
    wkeyT = nc.dram_tensor("wkeyT", [L, 128, DK], bf16, kind="ExternalInput").ap()
    bev_r = nc.dram_tensor("bev_r", [1, L * DZ], f32, kind="ExternalInput").ap()
    lng_r = nc.dram_tensor("lng_r", [1, L * DZ], f32, kind="ExternalInput").ap()
    lnb_r = nc.dram_tensor("lnb_r", [1, L * DZ], f32, kind="ExternalInput").ap()
    wg_r = nc.dram_tensor("wg_r", [1, L * DZ], f32, kind="ExternalInput").ap()
    bg_r = nc.dram_tensor("bg_r", [1, L], f32, kind="ExternalInput").ap()
    bval_r = nc.dram_tensor("bval_r", [1, L * DLVL], f32, kind="ExternalInput").ap()
    bkey_r = nc.dram_tensor("bkey_r", [1, L * DK], f32, kind="ExternalInput").ap()
    dec_r = nc.dram_tensor("dec_r", [1, L], f32, kind="ExternalInput").ap()

    outM = nc.dram_tensor("outM", [L, S, DLVL], f16, kind="ExternalOutput").ap()
    outK = nc.dram_tensor("outK", [L, S, DK], f16, kind="ExternalOutput").ap()

    with tile.TileContext(nc) as tc:
        with (
            tc.tile_pool(name="constp", bufs=1) as constp,
            tc.tile_pool(name="wp", bufs=1) as wp,
            tc.tile_pool(name="sm", bufs=1) as sm,
            tc.tile_pool(name="zrp", bufs=3) as zrp,
            tc.tile_pool(name="junkp", bufs=3) as junkp,
            tc.tile_pool(name="kresp", bufs=1) as kresp,
            tc.tile_pool(name="mip", bufs=B_MI) as mip,
            tc.tile_pool(name="eip", bufs=2) as eip,
            tc.tile_pool(name="pmisc", bufs=3, space="PSUM") as pmisc,
            tc.tile_pool(name="pkbp", bufs=2, space="PSUM") as pkbp,
            tc.tile_pool(name="pbcp", bufs=1, space="PSUM") as pbcp,
            tc.tile_pool(name="pmp", bufs=2, space="PSUM") as pmp,
            tc.tile_pool(name="dramp", bufs=1, space="DRAM") as dramp,
        ):
            # ---------------- constants / small input rows ----------------
            ident = constp.tile([128, 128], f32, name="ident")
            make_identity(nc, ident[:])
            identh = constp.tile([128, 128], f16, name="identh")
            nc.vector.tensor_copy(identh[:], ident[:])
            ones_row = constp.tile([1, 128], f32, name="ones_row")
            nc.gpsimd.memset(ones_row[:], 1.0)
            ones_col = constp.tile([128, 1], f32, name="ones_col")
            nc.gpsimd.memset(ones_col[:], 1.0)
            eps_sb = constp.tile([1, 1], f32, name="eps_sb")
            nc.gpsimd.memset(eps_sb[:], EPS)

            # per-level AR payload rows; memset BEFORE the dummy-AR trigger
            # occupies the gpsimd queue.
            z1s = [sm.tile([1, 8], f32, name=f"z1_{l}") for l in range(L)]
            for l in range(L):
                nc.gpsimd.memset(z1s[l][:], 0.0)
            z1d = sm.tile([1, 8], f32, name="z1d")
            nc.gpsimd.memset(z1d[:], 0.0)

            cc_ins = [dramp.tile([1, 8], f32, name=f"cc_in{l}")
                      for l in range(L)]
            cc_outs = [dramp.tile([1, 8], f32, name=f"cc_out{l}",
                                  addr_space="Shared") for l in range(L)]
            cc_ind = dramp.tile([1, 8], f32, name="cc_ind")
            cc_outd = dramp.tile([1, 8], f32, name="cc_outd",
                                 addr_space="Shared")

            def fire_ar(cin, cout):
                if not SKIP_CC:
                    nc.gpsimd.collective_compute(
                        "AllReduce", A.add,
                        replica_groups=[list(range(NCORES))],
                        ins=[cin.opt()], outs=[cout.opt()])
                else:
                    nc.gpsimd.dma_start(cout[:], cin[:])

            # dummy warm-up AllReduce: absorbs launch skew + cold-path cost
            # during the otherwise-dead head; the real ARs behind it on the
            # gpsimd queue then complete ~2.5us after their last trigger.
            nc.gpsimd.dma_start(cc_ind[:], z1d[:])
            fire_ar(cc_ind, cc_outd)

            def _row(name, src, width):
                t = sm.tile([1, width], f32, name=name)
                nc.sync.dma_start(t[:], src)
                return t

            bev_sb = _row("bev_sb", bev_r[:], L * DZ)
            lng_sb = _row("lng_sb", lng_r[:], L * DZ)
            lnb_sb = _row("lnb_sb", lnb_r[:], L * DZ)
            wg_sb = _row("wg_sb", wg_r[:], L * DZ)
            bg_sb = _row("bg_sb", bg_r[:], L)
            bval_sb = _row("bval_sb", bval_r[:], L * DLVL)
            bkey_sb = _row("bkey_sb", bkey_r[:], L * DK)
            dec_sb = _row("dec_sb", dec_r[:], L)

            # resident per-level K (keep-prescaled fp16): one DMA each on
            # the scalar ring. kres0 first (score L0 needs it ~20us in);
            # the tiny MLP weights ride between kres0 and kres1/2.
            kres = [kresp.tile([128, T, DK], f16, name=f"kres{l}")
                    for l in range(L)]
            nc.scalar.dma_start(
                kres[0][:], Kp[0].rearrange("(p t) d -> p t d", t=T))

            xcs, wevs, wvals, wkeys = [], [], [], []
            for l in range(L):
                xc = wp.tile([128, 14], bf16, name=f"xc{l}")
                nc.sync.dma_start(xc[:], xcatT[l])
                xcs.append(xc)
                wev = wp.tile([128, 14, 128], bf16, name=f"wev{l}")
                nc.sync.dma_start(
                    wev[:], wevT[l].rearrange("(c p) j -> p c j", p=128))
                wevs.append(wev)
                wval = wp.tile([128, DLVL], bf16, name=f"wval{l}")
                nc.scalar.dma_start(wval[:], wvalT[l])
                wvals.append(wval)
                wkey = wp.tile([128, DK], bf16, name=f"wkey{l}")
                nc.scalar.dma_start(wkey[:], wkeyT[l])
                wkeys.append(wkey)
            for l in (1, 2):
                nc.scalar.dma_start(
                    kres[l][:], Kp[l].rearrange("(p t) d -> p t d", t=T))

            # keep = 1 - decay; srk = SCALE / keep (folds the host-side
            # keep-prescale of K back out of the score dot product)
            keepr = sm.tile([1, L], f32, name="keepr")
            nc.scalar.activation(keepr[:], dec_sb[:], AF.Identity,
                                 bias=1.0, scale=-1.0)
            rk = sm.tile([1, L], f32, name="rk")
            nc.vector.reciprocal(rk[:], keepr[:])
            srk = sm.tile([1, L], f32, name="srk")
            nc.vector.tensor_scalar(srk[:], rk[:], SCALE, None, A.mult)

            # persistent per-level rows / broadcast tiles
            vrow = sm.tile([1, L * DLVL], f32, name="vrow")
            krow = sm.tile([1, L * DK], f32, name="krow")
            geff = sm.tile([1, L], f32, name="geff")
            inv = sm.tile([1, L], f32, name="inv")
            scores = sm.tile([128, L * T], f32, name="scores")
            zpart = sm.tile([128, L], f32, name="zpart")
            VBC = [sm.tile([128, DLVL], f16, name=f"VBC{l}") for l in range(L)]
            KBC = [sm.tile([128, DK], f16, name=f"KBC{l}") for l in range(L)]
            # PE update channel: unnormalized exp rows bounced through DRAM
            # (partition dim -> free dim) during the AR wait; rhs = v row
            # scaled by g/Z after the AR lands.
            ecr = dramp.tile([L * S], f16, name="ecr")
            vq = sm.tile([1, L * DLVL], f16, name="vq")
            pkbs = [None] * L

            # phase-2 M chunk loads (sync ring). Job j = (level j//NCH,
            # chunk j%NCH); the first B_MI are issued during phase 1.
            mis = {}

            def mi_load(j):
                l, c = divmod(j, NCH)
                mi = mip.tile([128, SUB, DLVL], f16, name="mi")
                nc.sync.dma_start(
                    mi[:],
                    Mp[l].rearrange("(p t) d -> p t d", t=T)
                    [:, c * SUB:(c + 1) * SUB, :])
                mis[j] = mi

            zS = [slice(l * DZ, (l + 1) * DZ) for l in range(L)]

            # -------- score-critical MLP chain: z -> LN -> relu -> k row ->
            # k broadcast. Gates this level's score pass / AR trigger.
            zcols = [None] * L
            zrows = [None] * L

            def mlp_crit(l):
                zsl = zS[l]
                ksl = slice(l * DK, (l + 1) * DK)
                pz = pmisc.tile([1, 128], f32, name="pz", tag="pmisc")
                for c in range(14):
                    nc.tensor.matmul(pz[:], lhsT=xcs[l][:, c:c + 1],
                                     rhs=wevs[l][:, c, :],
                                     start=(c == 0), stop=(c == 13))
                yield
                zr = zrp.tile([1, 128], f32, name="zr", bufs=2)
                nc.vector.tensor_tensor(zr[:], pz[:], bev_sb[:, zsl], op=A.add)
                musum = zrp.tile([1, 1], f32, name="musum", bufs=2)
                nc.vector.tensor_reduce(musum[:], zr[:], axis=AX.X, op=A.add)
                mu = zrp.tile([1, 1], f32, name="mu", bufs=2)
                nc.vector.tensor_scalar(mu[:], musum[:], 1.0 / DZ, None,
                                        A.mult)
                zm = zrp.tile([1, 128], f32, name="zm", bufs=2)
                nc.vector.tensor_scalar(zm[:], zr[:], mu[:], None, A.subtract)
                jr = junkp.tile([1, 128], f32, name="jr", bufs=2)
                nc.vector.tensor_tensor(jr[:], zm[:], zm[:], op=A.mult)
                vsum = zrp.tile([1, 1], f32, name="vsum", bufs=2)
                nc.vector.tensor_reduce(vsum[:], jr[:], axis=AX.X, op=A.add)
                yield
                std = zrp.tile([1, 1], f32, name="std", bufs=2)
                nc.scalar.activation(std[:], vsum[:], AF.Sqrt, bias=eps_sb[:],
                                     scale=1.0 / DZ)
                yield
                rstd = zrp.tile([1, 1], f32, name="rstd", bufs=2)
                nc.vector.reciprocal(rstd[:], std[:])
                zs2 = zrp.tile([1, 128], f32, name="zs2", bufs=2)
                nc.vector.scalar_tensor_tensor(
                    out=zs2[:], in0=zm[:], scalar=rstd[:],
                    in1=lng_sb[:, zsl], op0=A.mult, op1=A.mult)
                zs3 = zrp.tile([1, 128], f32, name="zs3", bufs=2)
                nc.vector.tensor_tensor(zs3[:], zs2[:], lnb_sb[:, zsl],
                                        op=A.add)
                zrow = sm.tile([1, 128], f32, name=f"zrow{l}")
                nc.vector.tensor_scalar(zrow[:], zs3[:], 0.0, None, A.max)
                zrows[l] = zrow
                yield
                pzc = pmisc.tile([128, 1], f32, name="pzc", tag="pmisc")
                nc.tensor.transpose(pzc[:], zrow[:], ident[0:1, 0:1])
                yield
                zcol = sm.tile([128, 1], bf16, name=f"zcol{l}")
                nc.vector.tensor_copy(zcol[:], pzc[:])
                zcols[l] = zcol
                yield
                pk0 = pmisc.tile([1, DK], f32, name="pk0", tag="pmisc")
                nc.tensor.matmul(pk0[:], lhsT=zcol[:], rhs=wkeys[l][:],
                                 start=True, stop=True)
                yield
                nc.vector.tensor_tensor(krow[:, ksl], pk0[:], bkey_sb[:, ksl],
                                        op=A.add)
                ksc = zrp.tile([1, DK], f32, name="ksc", bufs=2)
                nc.vector.tensor_scalar(ksc[:], krow[:, ksl],
                                        srk[:, l:l + 1], None, A.mult)
                yield
                pkb = pkbp.tile([128, DK], f32, name="pkb", tag="pkb")
                nc.tensor.matmul(pkb[:], lhsT=ones_row[:], rhs=ksc[:],
                                 start=True, stop=True)
                pkbs[l] = pkb

            # -------- non-critical tail: gate, v/tanh, VBC/KBC broadcasts.
            # Interleaved into the score pass; only consumed post-AR.
            def mlp_tail(l):
                zsl = zS[l]
                ksl = slice(l * DK, (l + 1) * DK)
                vsl = slice(l * DLVL, (l + 1) * DLVL)
                jg = junkp.tile([1, 128], f32, name="jg", bufs=2)
                nc.vector.tensor_tensor(jg[:], zrows[l][:], wg_sb[:, zsl],
                                        op=A.mult)
                gd = zrp.tile([1, 1], f32, name="gd", bufs=2)
                nc.vector.tensor_reduce(gd[:], jg[:], axis=AX.X, op=A.add)
                yield
                gsig = zrp.tile([1, 1], f32, name="gsig", bufs=2)
                nc.scalar.activation(gsig[:], gd[:], AF.Sigmoid,
                                     bias=bg_sb[:, l:l + 1], scale=1.0)
                yield
                msk = zrp.tile([1, 1], f32, name="msk", bufs=2)
                nc.vector.tensor_scalar(msk[:], gsig[:], THRESH, None,
                                        A.is_ge)
                nc.vector.tensor_tensor(geff[:, l:l + 1], gsig[:], msk[:],
                                        op=A.mult)
                yield
                pv = pmisc.tile([1, DLVL], f32, name="pv", tag="pmisc")
                nc.tensor.matmul(pv[:], lhsT=zcols[l][:], rhs=wvals[l][:],
                                 start=True, stop=True)
                yield
                vpre = zrp.tile([1, DLVL], f32, name="vpre", bufs=2)
                nc.vector.tensor_tensor(vpre[:], pv[:], bval_sb[:, vsl],
                                        op=A.add)
                yield
                nc.scalar.activation(vrow[:, vsl], vpre[:], AF.Tanh)
                yield
                pvb = pbcp.tile([128, DLVL], f32, name="pvb", tag="pbc")
                nc.tensor.matmul(pvb[:], lhsT=ones_row[:], rhs=vrow[:, vsl],
                                 start=True, stop=True)
                pkq = pbcp.tile([128, DK], f32, name="pkq", tag="pbc")
                nc.tensor.matmul(pkq[:], lhsT=ones_row[:], rhs=krow[:, ksl],
                                 start=True, stop=True)
                yield
                nc.vector.tensor_copy(VBC[l][:], pvb[:])
                nc.vector.tensor_copy(KBC[l][:], pkq[:])

            for _ in mlp_crit(0):
                pass
            # deep M prefetch: streams during the MLP/score/AR head.
            for j in range(B_MI):
                mi_load(j)

            # interleave schedule: score L0 carries crit(1), score L1
            # carries crit(2), score L2 carries all three tails (their
            # sigmoid/tanh ACT-table loads then sit BEHIND exp0/exp1 in the
            # ACT queue, keeping the L0/L1 AR trigger paths to Sqrt+Exp).
            for l in range(L):
                kr = kres[l]
                pkb = pkbs[l]
                if l == 0:
                    gens = [mlp_crit(1)]
                elif l == 1:
                    gens = [mlp_crit(2)]
                else:
                    gens = [mlp_tail(0), mlp_tail(1), mlp_tail(2)]
                cad = 6 if l < 2 else 3
                gi = 0

                def step():
                    nonlocal gi
                    while gi < len(gens):
                        if next(gens[gi], StopIteration) is StopIteration:
                            gi += 1
                        else:
                            return

                for j in range(T):
                    lt = l * T + j
                    jk = junkp.tile([128, 128], f16, name="jk", bufs=2)
                    nc.vector.scalar_tensor_tensor(
                        out=jk[:], in0=kr[:, j, :], scalar=1.0,
                        in1=pkb[:], op0=A.mult, op1=A.mult,
                        accum_out=scores[:, lt:lt + 1])
                    if j % cad == cad - 1:
                        step()
                nc.scalar.activation(scores[:, l * T:(l + 1) * T],
                                     scores[:, l * T:(l + 1) * T], AF.Exp)
                nc.vector.tensor_reduce(zpart[:, l:l + 1],
                                        scores[:, l * T:(l + 1) * T],
                                        axis=AX.X, op=A.add)
                pz1 = pmisc.tile([1, 1], f32, name="pz1", tag="pmisc")
                nc.tensor.matmul(pz1[:], lhsT=ones_col[:],
                                 rhs=zpart[:, l:l + 1], start=True, stop=True)
                nc.vector.tensor_copy(z1s[l][:, 0:1], pz1[:])
                nc.gpsimd.dma_start(cc_ins[l][:], z1s[l][:])
                fire_ar(cc_ins[l], cc_outs[l])
                # bounce the exp rows for the PE channel through DRAM
                # (overlaps the AR wait; only feeds ei loads ~20us later)
                pt = pmisc.tile([64, 128], f32, name="pt", tag="pmisc")
                nc.tensor.transpose(pt[:], scores[:, l * T:(l + 1) * T],
                                    ident[:])
                et = zrp.tile([64, 128], f16, name="et", bufs=1)
                nc.vector.tensor_copy(et[:], pt[:])
                nc.scalar.dma_start(
                    ecr[l * S:(l + 1) * S].rearrange("(t s) -> t s", s=128),
                    et[:])
                # flush leftover pipelined groups (non-critical from here)
                while gi < len(gens):
                    step()

            # ---- AR consumer for level l: zg (the AR wait) -> inv = g/Z ->
            # broadcast -> scale level-l exp scores in place. consumer(0)
            # runs before the first chunk; consumer(l+1) is hoisted into the
            # middle of level l's chunk loop.
            def ar_consumer(l):
                zg = zrp.tile([1, 8], f32, name="zg")
                nc.scalar.dma_start(zg[:], cc_outs[l][:])
                zrcp = zrp.tile([1, 1], f32, name="zrcp")
                nc.vector.reciprocal(zrcp[:], zg[:, 0:1])
                nc.vector.tensor_tensor(inv[:, l:l + 1], geff[:, l:l + 1],
                                        zrcp[:], op=A.mult)
                pib = pmisc.tile([128, 1], f32, name="pib", tag="pmisc")
                nc.tensor.matmul(pib[:], lhsT=ones_row[:],
                                 rhs=inv[:, l:l + 1], start=True, stop=True)
                ivb = zrp.tile([128, 1], f32, name="ivb")
                nc.vector.tensor_copy(ivb[:], pib[:])
                nc.vector.tensor_scalar(scores[:, l * T:(l + 1) * T],
                                        scores[:, l * T:(l + 1) * T],
                                        ivb[:], None, A.mult)
                # PE-channel rhs: v row scaled by g/Z (the exp rows it
                # multiplies stay unnormalized).
                vsl = slice(l * DLVL, (l + 1) * DLVL)
                nc.vector.tensor_scalar(vq[:, vsl], vrow[:, vsl],
                                        inv[:, l:l + 1], None, A.mult)

            # ei loads: the PE_N exp rows each chunk's PE channel needs,
            # as a single [1, PE_N*128] row on the scalar ring.
            def ei_load(l, c):
                ei = eip.tile([1, PE_N * 128], f16, name="ei", bufs=2)
                base = l * S + c * SUB * 128
                nc.scalar.dma_start(
                    ei[:],
                    ecr[base:base + PE_N * 128].rearrange(
                        "(a x) -> a x", a=1))
                return ei

            ar_consumer(0)
            H = SUB // 2
            eis = {c: ei_load(0, c) for c in range(3)}
            for l in range(L):
                kr = kres[l]
                vb = VBC[l]
                kb = KBC[l]
                vqs = vq[:, l * DLVL:(l + 1) * DLVL]
                outMv = outM[l].rearrange("(p t) d -> p t d", t=T)
                outKv = outK[l].rearrange("(p t) d -> p t d", t=T)
                for c in range(NCH):
                    cs = slice(c * SUB, (c + 1) * SUB)
                    mi = mis.pop(l * NCH + c)
                    ei = eis.pop(c)
                    if c + 3 < NCH:
                        eis[c + 3] = ei_load(l, c + 3)
                    elif l + 1 < L:
                        eis[c + 3 - NCH] = ei_load(l + 1, c + 3 - NCH)
                    for t in range(SUB):
                        j = c * SUB + t
                        sc = scores[:, l * T + j:l * T + j + 1]
                        if t < PE_N:
                            # PE+ACT channel: pm = exp_row (x) vq + I @ mi,
                            # ACT drains PSUM back into mi as fp16.
                            pm = pmp.tile([128, DLVL], f32, name="pm",
                                          tag="pm")
                            nc.tensor.matmul(
                                pm[:], lhsT=identh[:], rhs=mi[:, t, :],
                                start=True, stop=False)
                            nc.tensor.matmul(
                                pm[:], lhsT=ei[:, t * 128:(t + 1) * 128],
                                rhs=vqs, start=False, stop=True)
                            nc.scalar.copy(mi[:, t, :], pm[:])
                        else:
                            nc.vector.scalar_tensor_tensor(
                                out=mi[:, t, :], in0=vb[:], scalar=sc,
                                in1=mi[:, t, :], op0=A.mult, op1=A.add)
                        nc.vector.scalar_tensor_tensor(
                            out=kr[:, j, :], in0=kb[:], scalar=sc,
                            in1=kr[:, j, :], op0=A.mult, op1=A.add)
                        if t == H - 1:
                            # first-half M write overlaps second-half compute.
                            # M stores ride the sync ring: the ACT engine's
                            # per-trigger cost (~0.7us) was crowding out the
                            # PSUM drains.
                            nc.sync.dma_start(
                                outMv[:, c * SUB:c * SUB + H, :],
                                mi[:, 0:H, :])
                    nc.sync.dma_start(outMv[:, c * SUB + H:(c + 1) * SUB, :],
                                      mi[:, H:SUB, :])
                    nc.scalar.dma_start(outKv[:, cs, :], kr[:, cs, :])
                    # reuse this mi slot for the job B_MI ahead (emitted
                    # after this job's writes so the WAR is seen)
                    nj = l * NCH + c + B_MI
                    if nj < NJOB:
                        mi_load(nj)
                    if c == NCH - 2 and l + 1 < L:
                        ar_consumer(l + 1)

    nc.compile()
    return nc


def _prep_in_maps(inputs):
    f32 = np.float32
    f16 = np.float16
    import concourse.mybir as mybir
    bf16 = mybir.dt.np(mybir.dt.bfloat16)

    s_t = np.asarray(inputs["s_t"], f32)
    e_t = np.asarray(inputs["e_t"], f32)
    lc = np.asarray(inputs["level_contexts"], f32)
    W_ev0 = np.asarray(inputs["W_ev0"], f32)
    W_ev = np.asarray(inputs["W_ev"], f32)
    b_ev = np.asarray(inputs["b_ev"], f32)
    ln_g = np.asarray(inputs["ln_g"], f32)
    ln_b = np.asarray(inputs["ln_b"], f32)
    W_gate = np.asarray(inputs["W_gate"], f32)
    b_gate = np.asarray(inputs["b_gate"], f32)
    W_val = np.asarray(inputs["W_val"], f32)
    b_val = np.asarray(inputs["b_val"], f32)
    W_key = np.asarray(inputs["W_key"], f32)
    b_key = np.asarray(inputs["b_key"], f32)
    M = np.asarray(inputs["M"], f32)
    K = np.asarray(inputs["K"], f32)
    decay = np.asarray(inputs["decay"], f32)

    # unified MLP input per level: level 0 uses [s, e, 0-pad], levels 1-2 use
    # [s, ctx, e]; weight matrices padded/stacked to match.
    xcat = np.zeros((L, 1792), f32)
    xcat[0, :1024] = s_t
    xcat[0, 1024:1536] = e_t
    for l in (1, 2):
        xcat[l] = np.concatenate([s_t, lc[l - 1], e_t])
    xcatT = np.ascontiguousarray(
        xcat.reshape(L, 14, 128).transpose(0, 2, 1)).astype(bf16)
    W0p = np.concatenate([W_ev0, np.zeros((DZ, 256), f32)], axis=1)
    Wfull = np.stack([W0p, W_ev[0], W_ev[1]])
    wevT = np.ascontiguousarray(Wfull.transpose(0, 2, 1)).astype(bf16)
    wvalT = np.ascontiguousarray(W_val.transpose(0, 2, 1)).astype(bf16)
    wkeyT = np.ascontiguousarray(W_key.transpose(0, 2, 1)).astype(bf16)

    # fold keep = 1-decay into the M/K streams on the host (one f32 mult +
    # fp16 cast; same rounding count as casting raw M).
    keep = (1.0 - decay).astype(f32)[:, None, None]
    Mk = (keep * M).astype(f16)
    Kk = (keep * K).astype(f16)

    shared = dict(
        xcatT=xcatT, wevT=wevT, wvalT=wvalT, wkeyT=wkeyT,
        bev_r=b_ev.reshape(1, -1), lng_r=ln_g.reshape(1, -1),
        lnb_r=ln_b.reshape(1, -1), wg_r=W_gate.reshape(1, -1),
        bg_r=b_gate.reshape(1, -1), bval_r=b_val.reshape(1, -1),
        bkey_r=b_key.reshape(1, -1), dec_r=decay.reshape(1, -1),
    )
    in_maps = []
    for c in range(NCORES):
        sl = slice(c * S, (c + 1) * S)
        m = dict(shared)
        m["Mp"] = np.ascontiguousarray(Mk[:, sl, :])
        m["Kp"] = np.ascontiguousarray(Kk[:, sl, :])
        in_maps.append(m)
    return in_maps


def _run(inputs, trace=False):
    import concourse.bass_utils as bass_utils

    nc = _STATE.get("nc")
    if nc is None:
        nc = _build_bass()
        _STATE["nc"] = nc
    in_maps = _prep_in_maps(inputs)
    res = bass_utils.run_bass_kernel_spmd(
        nc, in_maps, core_ids=list(range(NCORES)), trace=trace)
    full = np.empty((L, N, DLVL + DK), np.float32)
    for c in range(NCORES):
        sl = slice(c * S, (c + 1) * S)
        full[:, sl, :DLVL] = res.results[c]["outM"]
        full[:, sl, DLVL:] = res.results[c]["outK"]
    return full, res


def kernel(**inputs):
    out, _ = _run(inputs, trace=False)
    return out
